# revision 29
# baseline (speedup 1.0000x reference)
"""Trainium2 Bass kernel for AttentionLSTM (2-layer LSTM + Bahdanau attention).

Sharding: data-parallel over batch. B=16 across 8 cores -> B_local=2 per core.
Each core runs the full 2-layer LSTM for its 2 batch rows, then the S x S
additive attention, entirely on-chip (the [B,S,S,A] intermediate is never
materialized in DRAM).

Layout notes (per core):
 - LSTM gates are computed transposed: PSUM [128 part = gate-slice, free = (m,b)]
   with M-tile (128-row blocks of the 4H=1024 gate dim) order
   [i0 i1 f0 f1 o0 o1 g0 g1]. The cell is tanh-only: sigmoid(x) =
   (tanh(x/2)+1)/2 with g-gate rows pre-doubled on host, so ONE tanh ACT
   covers all four gates; state is c~=2c / h~=2h with every h~-consuming
   weight halved on host (shortens the serial per-step chain by one ACT).
 - Hidden sequence h stored as hseq [128, 2(k), S, B] bf16 (h-transposed),
   which directly feeds next-step matmul rhs, the layer-1 input projection,
   and the attention projections.
 - Layer-0 hidden states and layer-1 input projections are chunked (CH=16
   steps) into separate tiles so the Tile scheduler pipelines layer 1 behind
   layer 0: each layer's serial PE->DVE->ACT chain overlaps the other
   layer's engine-idle time instead of running the layers back to back.
 - The gate PSUM tiles use PER-LAYER pool tags ("gates0"/"gates1"): pool
   slots are granted in allocation order per tag, so a shared tag would make
   layer 1's first PSUM allocation queue behind nearly all of layer 0's,
   serializing the layers completely (measured 1.32ms -> 0.80ms from this
   one change; the kernel is chain-latency-bound, not PE-bound).
 - Attention: uaT/waT [A=128 part, S free]; tanh arg built by a broadcast
   tensor_tensor add (0-step AP dims), tanh on ACT in [128, G*S] blocks,
   score reduction over A via per-i matmuls with lhsT=tanh tile, rhs=va
   column -> scores arrive transposed scT [j, i] in PSUM; PE-transpose to
   [i, j] for the row softmax.
"""

import numpy as np

B, S, D, H, A = 16, 256, 16, 256, 128
NCORES = 8
BL = B // NCORES  # 2 batch rows per core
G = 16            # attention i-group size

# M-tile slot order: i0 i1 f0 f1 o0 o1 g0 g1  (orig 4H blocks: i f g o)
MORDER = [0, 1, 2, 3, 6, 7, 4, 5]


def _build_bass():
    import concourse.bass as bass
    import concourse.bacc as bacc
    import concourse.mybir as mybir
    import concourse.tile as tile
    from concourse.masks import make_identity

    f32 = mybir.dt.float32
    bf16 = mybir.dt.bfloat16
    AF = mybir.ActivationFunctionType
    AX = mybir.AxisListType.X

    nc = bacc.Bacc("TRN2")

    # ---- DRAM tensors (inputs replicated except xT) ----
    d_xT = nc.dram_tensor("xT", [D, S * BL], bf16, kind="ExternalInput")
    d_whh0 = nc.dram_tensor("whh0", [128, 2 * 8 * 128], bf16, kind="ExternalInput")
    d_wih1 = nc.dram_tensor("wih1", [128, 2 * 8 * 128], bf16, kind="ExternalInput")
    d_whh1 = nc.dram_tensor("whh1", [128, 2 * 8 * 128], bf16, kind="ExternalInput")
    d_wih0 = nc.dram_tensor("wih0", [D, 8 * 128], bf16, kind="ExternalInput")
    d_bias0 = nc.dram_tensor("bias0", [128, 8], f32, kind="ExternalInput")
    d_bias1 = nc.dram_tensor("bias1", [128, 8], f32, kind="ExternalInput")
    d_waT = nc.dram_tensor("waT", [128, 2 * 128], bf16, kind="ExternalInput")
    d_uaT = nc.dram_tensor("uaT", [128, 2 * 128], bf16, kind="ExternalInput")
    d_attb = nc.dram_tensor("attb", [128, 1], f32, kind="ExternalInput")
    d_va = nc.dram_tensor("va", [128, 1], bf16, kind="ExternalInput")
    d_fcc = nc.dram_tensor("fcc", [128, 2], bf16, kind="ExternalInput")
    d_fch = nc.dram_tensor("fch", [128, 2], bf16, kind="ExternalInput")
    d_fcb = nc.dram_tensor("fcb", [1, 1], f32, kind="ExternalInput")

    d_attn = nc.dram_tensor("attn_out", [BL, S, S], f32, kind="ExternalOutput")
    d_out = nc.dram_tensor("out_vec", [BL, 1], f32, kind="ExternalOutput")

    with tile.TileContext(nc) as tc:
        import contextlib
        ctx = contextlib.ExitStack()
        with ctx:
            singles = ctx.enter_context(tc.tile_pool(name="singles", bufs=1))
            xpj = ctx.enter_context(tc.tile_pool(name="xpj", bufs=1))
            psg = ctx.enter_context(tc.tile_pool(name="psg", bufs=1, space="PSUM"))
            psb = ctx.enter_context(tc.tile_pool(name="psb", bufs=2, space="PSUM"))
            pssc = ctx.enter_context(tc.tile_pool(name="pssc", bufs=2, space="PSUM"))
            pstiny = ctx.enter_context(tc.tile_pool(name="pstiny", bufs=1, space="PSUM"))
            work = ctx.enter_context(tc.tile_pool(name="work", bufs=6))
            cst = ctx.enter_context(tc.tile_pool(name="cst", bufs=4))
            att = ctx.enter_context(tc.tile_pool(name="att", bufs=2))
            sm = ctx.enter_context(tc.tile_pool(name="sm", bufs=3))

            # ---- load weights into SBUF ----
            # Order matters: the xp0 GEMM (first PE work) needs only
            # wih0 + xT + bias0; then layer-0 steps need whh0. The big
            # layer-1 weights can trail behind the running recurrence.
            wih0 = singles.tile([D, 8, 128], bf16)
            nc.gpsimd.dma_start(out=wih0, in_=d_wih0[:].rearrange("p (m c) -> p m c", m=8))
            xT = singles.tile([D, S * BL], bf16)
            nc.gpsimd.dma_start(out=xT, in_=d_xT[:])
            bias0 = singles.tile([128, 8], f32)
            bias1 = singles.tile([128, 8], f32)
            nc.gpsimd.dma_start(out=bias0, in_=d_bias0[:])
            whh0 = singles.tile([128, 2, 8, 128], bf16)
            nc.gpsimd.dma_start(out=whh0, in_=d_whh0[:].rearrange("p (k m c) -> p k m c", k=2, m=8))
            wih1 = singles.tile([128, 2, 8, 128], bf16)
            whh1 = singles.tile([128, 2, 8, 128], bf16)
            nc.gpsimd.dma_start(out=wih1, in_=d_wih1[:].rearrange("p (k m c) -> p k m c", k=2, m=8))
            nc.gpsimd.dma_start(out=whh1, in_=d_whh1[:].rearrange("p (k m c) -> p k m c", k=2, m=8))
            nc.gpsimd.dma_start(out=bias1, in_=d_bias1[:])
            waT = singles.tile([128, 2, 128], bf16)
            uaT = singles.tile([128, 2, 128], bf16)
            nc.gpsimd.dma_start(out=waT, in_=d_waT[:].rearrange("p (k c) -> p k c", k=2))
            nc.gpsimd.dma_start(out=uaT, in_=d_uaT[:].rearrange("p (k c) -> p k c", k=2))
            attb = singles.tile([128, 1], f32)
            nc.gpsimd.dma_start(out=attb, in_=d_attb[:])
            va = singles.tile([128, 1], bf16)
            nc.gpsimd.dma_start(out=va, in_=d_va[:])
            fcc = singles.tile([128, 2], bf16)
            fch = singles.tile([128, 2], bf16)
            nc.gpsimd.dma_start(out=fcc, in_=d_fcc[:])
            nc.gpsimd.dma_start(out=fch, in_=d_fch[:])
            fcb = singles.tile([1, 1], f32)
            nc.gpsimd.dma_start(out=fcb, in_=d_fcb[:])

            ident = singles.tile([128, 128], f32)
            make_identity(nc, ident)

            zh = singles.tile([128, 2, 1, BL], bf16)   # zero h for step 0
            nc.vector.memset(zh, 0.0)
            zc = singles.tile([128, 2, BL], f32)       # zero c for step 0
            nc.vector.memset(zc, 0.0)

            # hidden sequences (h-transposed): [128, k, S, B]
            CH = 16  # layer-0 -> layer-1 pipeline chunk (steps)
            NCH = S // CH
            h1c = [singles.tile([128, 2, CH, BL], bf16, tag=f"h1c{i}", name=f"h1c{i}") for i in range(NCH)]
            h2c = [singles.tile([128, 2, CH, BL], bf16, tag=f"h2c{i}", name=f"h2c{i}") for i in range(NCH)]
            h2full = singles.tile([128, 2, S, BL], bf16)

            def input_proj(wT, rhs_k, nk, biast, kdim):
                """xproj[:, m, s, b] = sum_k wT[:,k,m,:].T @ rhs_k(k) + bias"""
                xp = xpj.tile([128, 8, S, BL], f32, tag="xproj")
                for m in range(8):
                    ps = psb.tile([128, S * BL], f32, tag="gemm")
                    for k in range(nk):
                        nc.tensor.matmul(
                            ps,
                            wT[:kdim, k, m, :] if nk > 1 else wT[:kdim, m, :],
                            rhs_k(k),
                            start=(k == 0),
                            stop=(k == nk - 1),
                        )
                    nc.scalar.activation(
                        xp[:, m, :, :].rearrange("p s b -> p (s b)"),
                        ps, AF.Identity, bias=biast[:, m : m + 1],
                    )
                return xp

            def lstm_layer(get_xp, whh, get_h, put_h, ltag):
                for t in range(S):
                    psum_g = psg.tile([128, 8, BL], f32, tag="gates" + ltag)
                    xp3 = get_xp(t)
                    for m in range(8):
                        # inject the precomputed input projection into PSUM via
                        # an identity matmul so ACT can read gates directly from
                        # PSUM -- removes the DVE add from the serial chain
                        nc.tensor.matmul(
                            psum_g[:, m, :], ident, xp3[:, m, :],
                            start=True, stop=False,
                        )
                        for k in range(2):
                            rhs = zh[:, k, 0, :] if t == 0 else get_h(t - 1, k)
                            nc.tensor.matmul(
                                psum_g[:, m, :],
                                whh[:, k, m, :],
                                rhs,
                                start=False,
                                stop=(k == 1),
                            )
                    gsum = psum_g
                    # tanh-only rewrite: sigmoid(x) = (tanh(x/2)+1)/2 with
                    # g-gate rows pre-doubled on host; state is c~=2c, h~=2h
                    # (all h~ consumers halved on host).
                    acts = work.tile([128, 8, BL], f32, tag="acts" + ltag)
                    nc.scalar.activation(acts, gsum, AF.Tanh, scale=0.5)
                    ADD, MUL = mybir.AluOpType.add, mybir.AluOpType.mult
                    t1 = work.tile([128, 2, BL], f32, tag="t1" + ltag)
                    nc.vector.scalar_tensor_tensor(
                        t1, acts[:, 0:2, :], 1.0, acts[:, 6:8, :], op0=ADD, op1=MUL)
                    m2 = work.tile([128, 2, BL], f32, tag="m2" + ltag)
                    c_old = zc if t == 0 else c_prev
                    nc.vector.scalar_tensor_tensor(
                        m2, acts[:, 2:4, :], 1.0, c_old, op0=ADD, op1=MUL)
                    c_new = cst.tile([128, 2, BL], f32, tag="c" + ltag)
                    nc.vector.scalar_tensor_tensor(
                        c_new, m2, 0.5, t1, op0=MUL, op1=ADD)
                    thc = work.tile([128, 2, BL], f32, tag="thc" + ltag)
                    nc.scalar.activation(thc, c_new, AF.Tanh, scale=0.5)
                    nc.vector.scalar_tensor_tensor(
                        put_h(t), acts[:, 4:6, :], 1.0, thc, op0=ADD, op1=MUL)
                    c_prev = c_new

            # ---- layer 0 ----
            xp0 = input_proj(wih0, lambda k: xT, 1, bias0, D)
            lstm_layer(
                lambda t: xp0[:, :, t, :], whh0,
                lambda t, k: h1c[t // CH][:, k, t % CH, :],
                lambda t: h1c[t // CH][:, :, t % CH, :],
                "0",
            )
            # ---- layer 1: per-chunk input projection GEMMs ----
            xp1c = []
            for ci in range(NCH):
                xpc = xpj.tile([128, 8, CH, BL], f32, tag=f"xp1c{ci}", name=f"xp1c{ci}")
                for m in range(8):
                    ps = psb.tile([128, CH * BL], f32, tag="gemm")
                    for k in range(2):
                        nc.tensor.matmul(
                            ps,
                            wih1[:, k, m, :],
                            h1c[ci][:, k, :, :].rearrange("p s b -> p (s b)"),
                            start=(k == 0),
                            stop=(k == 1),
                        )
                    nc.scalar.activation(
                        xpc[:, m, :, :].rearrange("p s b -> p (s b)"),
                        ps, AF.Identity, bias=bias1[:, m : m + 1],
                    )
                xp1c.append(xpc)
            lstm_layer(
                lambda t: xp1c[t // CH][:, :, t % CH, :], whh1,
                lambda t, k: h2c[t // CH][:, k, t % CH, :],
                lambda t: h2c[t // CH][:, :, t % CH, :],
                "1",
            )
            # assembled copy of h2 for the small output-path matmuls
            for ci in range(NCH):
                nc.vector.tensor_copy(h2full[:, :, ci * CH : (ci + 1) * CH, :], h2c[ci])

            # ---- attention, per local batch ----
            # j-half structure: the j<128 half of the tanh/score grid only
            # needs h2 chunks 0..7 + wa rows for its i-group, so it runs
            # during the tail of the LSTM; only the j>=128 half waits for
            # the full sequence.
            import concourse.bass as bassmod

            for b in range(BL):
                # wa per i-chunk tiles (attb folded into the sums add below)
                wa_c = []
                for ci in range(NCH):
                    wap = psb.tile([128, CH], f32, tag="gemm", name=f"wap{b}_{ci}")
                    for k in range(2):
                        nc.tensor.matmul(wap, waT[:, k, :], h2c[ci][:, k, :, b],
                                         start=(k == 0), stop=(k == 1))
                    wc = att.tile([128, CH], bf16, tag=f"wac{b}_{ci}", name=f"wac{b}_{ci}")
                    nc.vector.tensor_copy(wc, wap)
                    wa_c.append(wc)

                scT_ps = pssc.tile([128, 2, S], f32, tag="scT", name=f"scT{b}")
                ua_jb = []
                for jb in range(2):
                    uap = psb.tile([128, 128], f32, tag="gemm", name=f"uap{b}_{jb}")
                    for c8 in range(8):
                        ci = jb * 8 + c8
                        for k in range(2):
                            nc.tensor.matmul(
                                uap[:, c8 * CH : (c8 + 1) * CH],
                                uaT[:, k, :], h2c[ci][:, k, :, b],
                                start=(k == 0), stop=(k == 1),
                            )
                    u = att.tile([128, 128], bf16, tag=f"ua{b}_{jb}", name=f"ua{b}_{jb}")
                    nc.vector.tensor_copy(u, uap)
                    ua_jb.append(u)

                    for g in range(S // G):
                        sums = sm.tile([128, G, 128], bf16, tag=f"sums{b}", name=f"sums{b}_{jb}_{g}")
                        wa_sl = wa_c[(g * G) // CH][:, (g * G) % CH : (g * G) % CH + G]
                        wa_bc = bassmod.AP(
                            tensor=wa_sl.tensor, offset=wa_sl.offset,
                            ap=[*wa_sl.ap, [0, 128]],
                        )
                        ua_full = u[:]
                        ua_bc = bassmod.AP(
                            tensor=ua_full.tensor, offset=ua_full.offset,
                            ap=[ua_full.ap[0], [0, G], ua_full.ap[1]],
                        )
                        nc.vector.scalar_tensor_tensor(
                            sums, ua_bc, attb, wa_bc,
                            op0=mybir.AluOpType.add, op1=mybir.AluOpType.add,
                        )
                        th = sm.tile([128, G, 128], bf16, tag=f"th{b}", name=f"th{b}_{jb}_{g}")
                        nc.scalar.activation(th, sums, AF.Tanh)
                        for ii in range(G):
                            i = g * G + ii
                            nc.tensor.matmul(
                                scT_ps[:, jb, i : i + 1],
                                th[:, ii, :], va,
                                start=True, stop=True,
                            )
                scT_sb = att.tile([128, 2, S], f32, tag="scTsb", name=f"scTsb{b}")
                nc.vector.tensor_copy(scT_sb, scT_ps)

                # transpose score blocks and row-softmax
                for ib in range(2):
                    scp = pstiny.tile([128, 2, 128], f32, tag="tiny", name=f"scp{b}_{ib}")
                    for jt in range(2):
                        nc.tensor.transpose(
                            scp[:, jt, :],
                            scT_sb[:, jt, ib * 128 : (ib + 1) * 128],
                            ident,
                        )
                    nmx = work.tile([128, 1], f32, tag="nmx", name=f"nmx{b}_{ib}")
                    nc.vector.tensor_reduce(
                        nmx, scp.rearrange("p a b -> p (a b)"),
                        axis=AX, op=mybir.AluOpType.max, negate=True,
                    )
                    esb = att.tile([128, S], f32, tag="esb", name=f"esb{b}_{ib}")
                    nc.scalar.activation(
                        esb, scp.rearrange("p a b -> p (a b)"), AF.Exp, bias=nmx
                    )
                    ssum = work.tile([128, 1], f32, tag="ssum", name=f"ssum{b}_{ib}")
                    nc.vector.reduce_sum(ssum, esb, axis=AX)
                    rcp = work.tile([128, 1], f32, tag="rcp", name=f"rcp{b}_{ib}")
                    nc.vector.reciprocal(rcp, ssum)
                    atn = att.tile([128, S], f32, tag="atn", name=f"atn{b}_{ib}")
                    nc.vector.tensor_scalar_mul(atn, esb, rcp)
                    nc.gpsimd.dma_start(
                        out=d_attn[b, ib * 128 : (ib + 1) * 128, :], in_=atn
                    )

                # final projection output
                ec = work.tile([128, 2], f32, tag="ec", name=f"ec{b}")
                nc.scalar.activation(ec, scT_sb[:, :, S - 1], AF.Exp)
                q_ps = pstiny.tile([128, 2], f32, tag="tiny", name=f"qps{b}")
                for jt in range(2):
                    for k in range(2):
                        nc.tensor.matmul(
                            q_ps[:, jt : jt + 1],
                            h2full[:, k, jt * 128 : (jt + 1) * 128, b],
                            fcc[:, k : k + 1],
                            start=(k == 0), stop=(k == 1),
                        )
                qo = work.tile([128, 2, 2], f32, tag="qo", name=f"qo{b}")
                nc.vector.memset(qo, 1.0)
                nc.vector.tensor_copy(qo[:, :, 0], q_ps)
                s1_ps = pstiny.tile([128, 2], f32, tag="tiny", name=f"s1ps{b}")
                for jt in range(2):
                    nc.tensor.matmul(
                        s1_ps[0:1, :],
                        ec[:, jt : jt + 1],
                        qo[:, jt, :],
                        start=(jt == 0), stop=(jt == 1),
                    )
                fc_ps = pstiny.tile([128, 1], f32, tag="tiny", name=f"fcps{b}")
                for k in range(2):
                    nc.tensor.matmul(
                        fc_ps[0:1, :],
                        h2full[:, k, S - 1 : S, b],
                        fch[:, k : k + 1],
                        start=(k == 0), stop=(k == 1),
                    )
                r1 = work.tile([1, 1], f32, tag="r1", name=f"r1{b}")
                nc.vector.reciprocal(r1, s1_ps[0:1, 1:2])
                o1 = work.tile([1, 1], f32, tag="o1", name=f"o1{b}")
                nc.vector.tensor_mul(o1, s1_ps[0:1, 0:1], r1)
                nc.vector.tensor_add(o1, o1, fc_ps[0:1, :])
                nc.vector.tensor_add(o1, o1, fcb)
                nc.gpsimd.dma_start(out=d_out[b : b + 1, :], in_=o1)

    nc.compile()
    return nc


def _host_prep(inputs):
    """Build per-core input maps (numpy, host-side reshapes/casts only)."""
    import ml_dtypes

    bf16 = ml_dtypes.bfloat16
    x = np.asarray(inputs["x"], np.float32)

    def mreord(w4h):  # reorder rows of a [4H, K] matrix into MORDER 128-blocks
        blocks = w4h.reshape(8, 128, -1)
        return blocks[MORDER].reshape(1024, -1)

    def gdouble(w4h):  # double the g-gate block rows (tanh-only LSTM rewrite)
        w = np.array(w4h, np.float32)
        w[512:768] *= 2.0
        return w

    def pack_whh(w):  # [4H=1024, 256] -> lhsT [128, 2(k), 8(m), 128(c)] (bf16)
        wr = mreord(w)            # rows reordered
        wT = wr.T                 # [256(h), 1024(g)]
        t = wT.reshape(2, 128, 8, 128)   # [k, p, m, c]
        return np.ascontiguousarray(t.transpose(1, 0, 2, 3)).reshape(128, -1).astype(bf16)

    whh0 = pack_whh(0.5 * gdouble(inputs["W_hh0"]))
    wih1 = pack_whh(0.5 * gdouble(inputs["W_ih1"]))
    whh1 = pack_whh(0.5 * gdouble(inputs["W_hh1"]))
    wih0 = np.ascontiguousarray(mreord(gdouble(inputs["W_ih0"])).T).astype(bf16)  # [16, 1024]

    def breord(bv):
        return bv.reshape(8, 128)[MORDER]  # [8, 128]

    bias0 = np.ascontiguousarray(
        breord(gdouble((np.asarray(inputs["b_ih0"], np.float32)
                        + np.asarray(inputs["b_hh0"], np.float32)).reshape(-1, 1))[:, 0]).T
    ).astype(np.float32)  # [128, 8]
    bias1 = np.ascontiguousarray(
        breord(gdouble((np.asarray(inputs["b_ih1"], np.float32)
                        + np.asarray(inputs["b_hh1"], np.float32)).reshape(-1, 1))[:, 0]).T
    ).astype(np.float32)

    def pack_att(wm):  # [A=128, H=256] -> lhsT [128(p=h%128), 2(k), 128(a)]
        t = wm.T.reshape(2, 128, 128)  # [k, h, a]
        return np.ascontiguousarray(t.transpose(1, 0, 2)).reshape(128, 256).astype(bf16)

    waT = pack_att(0.5 * np.asarray(inputs["Wa_w"], np.float32))
    uaT = pack_att(0.5 * np.asarray(inputs["Ua_w"], np.float32))
    attb = (np.asarray(inputs["Wa_b"], np.float32) + np.asarray(inputs["Ua_b"], np.float32)).reshape(128, 1)
    va = np.asarray(inputs["va_w"], np.float32).reshape(128, 1).astype(bf16)
    fcw = np.asarray(inputs["fc_w"], np.float32)[0]  # [512]
    fcc = np.ascontiguousarray((0.5 * fcw[:256]).reshape(2, 128).T).astype(bf16)  # [128, 2]
    fch = np.ascontiguousarray((0.5 * fcw[256:]).reshape(2, 128).T).astype(bf16)
    fcb = np.asarray(inputs["fc_b"], np.float32).reshape(1, 1)

    shared = dict(
        whh0=whh0, wih1=wih1, whh1=whh1, wih0=wih0,
        bias0=bias0, bias1=bias1, waT=waT, uaT=uaT, attb=attb,
        va=va, fcc=fcc, fch=fch, fcb=fcb,
    )
    in_maps = []
    for c in range(NCORES):
        xs = x[c * BL : (c + 1) * BL]                # [2, S, D]
        xT = np.ascontiguousarray(xs.transpose(2, 1, 0).reshape(D, S * BL)).astype(bf16)
        in_maps.append(dict(shared, xT=xT))
    return in_maps


_CACHED = {}


def _get_nc():
    if "nc" not in _CACHED:
        _CACHED["nc"] = _build_bass()
    return _CACHED["nc"]


def run(inputs, trace=False):
    from concourse.bass_utils import run_bass_kernel_spmd

    nc = _get_nc()
    in_maps = _host_prep(inputs)
    res = run_bass_kernel_spmd(nc, in_maps, core_ids=list(range(NCORES)), trace=trace)
    attn = np.concatenate([r["attn_out"] for r in res.results], axis=0).astype(np.float32)
    out = np.concatenate([r["out_vec"] for r in res.results], axis=0).astype(np.float32)
    return (out, attn), res


def kernel(**inputs):
    (out, attn), _ = run(inputs)
    return (out, attn)
